# revision 33
# baseline (speedup 1.0000x reference)
"""Trainium2 Bass kernel for EnhancedFastKANLayer.

Reference computation (B=16384, D=O=512, G=8 grids):
    x_norm = (x - mean) * rsqrt(var + eps) * gamma + beta          # BN inference
    basis[b,d,g] = exp(-((x_norm[b,d] - grid[g]) / denom)^2)       # RBF expansion
    out = basis.reshape(B, D*G) @ W_spline + b_spline
        + relu(x) @ W_base + b_base + x

Strategy (v3):
  - Data parallel: batch 16384 sharded 8 ways (2048 rows/core); weights
    replicated. No collectives.
  - All on-chip compute in transposed layout [feature, batch]; output is
    produced as out_T [O, B_shard] fp16 and combined on the host: the host
    adds the +x residual and the (b_spline+b_base) bias during the gather
    pass, so the chip runs only the 36 real K-chunks (4 base + 32 spline).
  - RBF via ScalarE Derivative_Erf: basis_g = sqrt(pi)/2 *
    Derivative_Erf(uscale_d*x + (ushift_d - c_g)): ONE ACT op per
    (d-tile, grid) directly from the fp16 x tile (BN affine folded into
    ACT's per-partition scale/bias; sqrt(pi)/2 folded into W_spline).
  - x pre-cast to fp16 and pre-transposed to [D, B_shard] on the host.
  - Matmul: kc-OUTER sweep; the 4 BASE K-chunks go FIRST -- relu(x) needs
    only the x tile (no params, no ACT), so the real matmul stream starts
    ~1.5us after the first x DMA lands, and the spline/weight DMA deadlines
    all relax by the ~7us the base chunks take.  Each basis tile is dead
    right after its 8 matmuls (small pool).  The last 6 spline K-chunks run
    per-bank, fused with the PSUM->SBUF fp16 copy + out DMA so bank
    epilogues pipeline with remaining matmuls.
  - Startup: critical-first DMA order (sync: x d-tile 0 in halves and
    W_base d-slice 0; gpsimd: ACT params then the bulk W_spline stream);
    N=512 warmup matmuls on a zeroed tile bridge the HAM clock-throttle
    window until the first base matmul.
"""

import numpy as np
from contextlib import ExitStack

import concourse.bass as bass
import concourse.tile as tile
from concourse import bacc, mybir
from concourse._compat import with_exitstack
from concourse.bass_utils import run_bass_kernel_spmd

N_CORES = 8
BATCH, IN_DIM, OUT_DIM, G = 16384, 512, 512, 8
B_SHARD = BATCH // N_CORES          # 2048
B_CHUNK = 1024                      # batch columns processed per chunk
GRID_MIN, GRID_MAX, BN_EPS = -2.0, 2.0, 1e-3
DENOM = (GRID_MAX - GRID_MIN) / G   # 0.5
N_DT = IN_DIM // 128                # 4 d-tiles
K_SPLINE = N_DT * G                 # 32 spline K-chunks
K_BASE = N_DT                       # 4 base K-chunks
N_OSUB = OUT_DIM // 128             # 4 output partition tiles

F32 = mybir.dt.float32
F16 = mybir.dt.float16


def _grid_consts():
    grid = np.linspace(GRID_MIN, GRID_MAX, G, dtype=np.float32)
    c = (grid / np.float32(DENOM)).astype(np.float32)        # grid in u-units
    return c


@with_exitstack
def _body(ctx, tc, x16t, w_sp, w_b, params, out_t, b_shard, b_chunk):
    nc = tc.nc
    n_chunks = b_shard // b_chunk
    n_bh = b_chunk // 512            # 512-wide moving-operand slices
    # K-chunk schedule: 4 base chunks first, then 32 spline chunks; the
    # last N_FINAL spline chunks run per-bank fused with the epilogue.
    N_FINAL = 6
    sweep_kcs = list(range(K_SPLINE, K_SPLINE + K_BASE)) + \
        list(range(K_SPLINE - N_FINAL))
    final_kcs = list(range(K_SPLINE - N_FINAL, K_SPLINE))
    N_WARM = 22                      # N=128 junk matmuls to lift HAM throttle

    const_pool = ctx.enter_context(tc.tile_pool(name="const", bufs=1))
    w_pool = ctx.enter_context(tc.tile_pool(name="w", bufs=1))
    xt_pool = ctx.enter_context(tc.tile_pool(name="xt", bufs=6))
    basis_pool = ctx.enter_context(tc.tile_pool(name="basis", bufs=8))
    relu_pool = ctx.enter_context(tc.tile_pool(name="relu", bufs=4))
    psum_pool = ctx.enter_context(
        tc.tile_pool(name="psum", bufs=8, space="PSUM"))
    out_pool = ctx.enter_context(tc.tile_pool(name="outs", bufs=3))

    # ---- ACT table preload: walrus inserts the ACT_TABLE_LOAD before this
    # first ACTIVATE so it overlaps the input DMAs ----
    # zeros tile: feeds the PE warm-up matmuls, and its first column doubles
    # as the scratch input for the ACT-table preload; the scratch output
    # tile is later reused as the bulk-DMA gate (WAW-ordered).  Memset on
    # gpsimd: its preamble ends ~0.7us before vector's, so the warmup (and
    # the HAM un-throttle window it drives) starts that much earlier.
    zeros = const_pool.tile([128, 128], F16)
    nc.gpsimd.memset(zeros, 0.0)
    scratch = const_pool.tile([128, 1], F16)
    nc.scalar.activation(out=scratch, in_=zeros[:, 0:1],
                         func=mybir.ActivationFunctionType.Derivative_Erf)

    # ---- critical-first DMA order.
    # sync (HWDGE): x d-tile 0 in halves + W_base d-slice 0, then the rest
    # of W_base, the remaining x tiles and W_spline K-chunks 0-1.
    # gpsimd (SWDGE): ACT params, then the bulk W_spline stream (emitted
    # after chunk-0 producers so its issues don't crowd the criticals).
    params_sb = const_pool.tile([128, N_DT + N_DT * G], F32)
    nc.gpsimd.dma_start(out=params_sb, in_=params)
    uscale_sb = params_sb[:, 0:N_DT]                    # per-d ACT scale
    abias_sb = params_sb[:, N_DT:]                      # per-(d,g) ACT bias
    # single weight tile: spline K-chunks 0..31, base chunks at 32..35
    w_all = w_pool.tile([128, K_SPLINE + K_BASE, OUT_DIM], F16)
    w_tile = w_all[:, 0:K_SPLINE, :]
    wb_tile = w_all[:, K_SPLINE:, :]

    # sync/HWDGE queue, ordered by consumption deadline (SWDGE latency is
    # ~5us, so every startup-critical transfer must ride the sync FIFO)
    xt0 = xt_pool.tile([128, b_chunk], F16, tag="xt")
    nc.sync.dma_start(out=xt0[:, 0:512], in_=x16t[0:128, 0:512])
    nc.sync.dma_start(out=wb_tile[:, 0:1, :], in_=w_b[:, 0:1, :])
    nc.sync.dma_start(out=xt0[:, 512:1024], in_=x16t[0:128, 512:1024])
    xt1 = xt_pool.tile([128, b_chunk], F16, tag="xt")
    nc.sync.dma_start(out=xt1, in_=x16t[128:256, 0:b_chunk])
    nc.sync.dma_start(out=wb_tile[:, 1:2, :], in_=w_b[:, 1:2, :])
    xt2 = xt_pool.tile([128, b_chunk], F16, tag="xt")
    nc.sync.dma_start(out=xt2, in_=x16t[256:384, 0:b_chunk])
    nc.sync.dma_start(out=wb_tile[:, 2:K_BASE, :], in_=w_b[:, 2:K_BASE, :])
    xt3 = xt_pool.tile([128, b_chunk], F16, tag="xt")
    nc.sync.dma_start(out=xt3, in_=x16t[384:512, 0:b_chunk])
    nc.sync.dma_start(out=w_tile[:, 0:2, :], in_=w_sp[:, 0:2, :])
    xts0 = [xt0, xt1, xt2, xt3]

    def emit_producers(ch, xts=None):
        b0 = ch * b_chunk
        relus, basis = [], []
        for dt in range(N_DT):
            if xts is not None:
                xt = xts[dt]
            else:
                xt = xt_pool.tile([128, b_chunk], F16, tag="xt")
                nc.sync.dma_start(
                    out=xt,
                    in_=x16t[dt * 128:(dt + 1) * 128, b0:b0 + b_chunk],
                )
            rl = relu_pool.tile([128, b_chunk], F16, tag="relu")
            if ch == 0 and dt == 0:
                # halves: the first base matmuls depend only on half 0
                nc.vector.tensor_scalar_max(
                    out=rl[:, 0:512], in0=xt[:, 0:512], scalar1=0.0)
                nc.vector.tensor_scalar_max(
                    out=rl[:, 512:1024], in0=xt[:, 512:1024], scalar1=0.0)
            else:
                nc.vector.tensor_scalar_max(out=rl, in0=xt, scalar1=0.0)
            relus.append(rl)
            for g in range(G):
                bt = basis_pool.tile([128, b_chunk], F16, tag="basis")
                # basis_g = sqrt(pi)/2 * d/dx erf(uscale*x + ushift - c_g);
                # constant folded into W_spline host-side.
                nc.scalar.activation(
                    out=bt, in_=xt,
                    func=mybir.ActivationFunctionType.Derivative_Erf,
                    bias=abias_sb[:, dt * G + g:dt * G + g + 1],
                    scale=uscale_sb[:, dt:dt + 1],
                )
                basis.append(bt)
        return relus, basis

    def operands(kc, osub, relus, basis):
        if kc < K_SPLINE:
            return w_tile[:, kc, osub * 128:(osub + 1) * 128], basis[kc]
        dt = kc - K_SPLINE
        return wb_tile[:, dt, osub * 128:(osub + 1) * 128], relus[dt]

    def emit_main_sweep(ch, psums, relus, basis):
        for idx, kc in enumerate(sweep_kcs):
            # bh-outer on the very first chunk's first K-chunk so the first
            # 4 matmuls depend only on the first relu half-tile.
            order = ([(osub, bh) for bh in range(n_bh)
                      for osub in range(N_OSUB)]
                     if (ch == 0 and idx == 0) else
                     [(osub, bh) for osub in range(N_OSUB)
                      for bh in range(n_bh)])
            for osub, bh in order:
                lhsT, rhs = operands(kc, osub, relus, basis)
                nc.tensor.matmul(
                    psums[osub * n_bh + bh], lhsT=lhsT,
                    rhs=rhs[:, bh * 512:(bh + 1) * 512],
                    start=(idx == 0), stop=False)

    def emit_final_blocks(ch, psums, relus, basis):
        b0 = ch * b_chunk
        for osub in range(N_OSUB):
            for bh in range(n_bh):
                ps = psums[osub * n_bh + bh]
                for j, kc in enumerate(final_kcs):
                    lhsT, rhs = operands(kc, osub, relus, basis)
                    nc.tensor.matmul(
                        ps, lhsT=lhsT, rhs=rhs[:, bh * 512:(bh + 1) * 512],
                        start=False, stop=(j == N_FINAL - 1))
                ot = out_pool.tile([128, 512], F16, tag="ot")
                nc.vector.tensor_scalar_add(out=ot, in0=ps, scalar1=0.0)
                nc.sync.dma_start(
                    out=out_t[osub * 128:(osub + 1) * 128,
                              b0 + bh * 512:b0 + (bh + 1) * 512],
                    in_=ot)

    def alloc_psums(ch):
        return [psum_pool.tile([128, 512], F32, tag="ps", name=f"ps{ch}_{i}")
                for i in range(N_OSUB * n_bh)]

    psums0 = alloc_psums(0)
    # PE warm-up: dependency-free N=128 matmuls (107ns cold apiece -- fine
    # bridge quantum) release the HAM clock throttle (~3.4us of sustained
    # PE activity) while the first x tile lands; they target the
    # LAST-consumed bank, and the real first matmul there has start=True
    # which resets it, so junk never reaches the output.
    for j in range(N_WARM):
        nc.tensor.matmul(psums0[N_OSUB * n_bh - 1][:, 0:128], lhsT=zeros,
                         rhs=zeros, start=(j == 0), stop=(j == N_WARM - 1))

    prod = emit_producers(0, xts0)
    # bulk W_spline stream (gpsimd), gated behind the last critical sync
    # transfer (W_spline K-chunks 0-1) so the DMA rings stay clear for the
    # latency-critical startup transfers; the bulk has ~10us of slack
    # before K-chunk 2 is consumed.  The Tile scheduler reorders engine
    # streams by readiness, so the gating must be DATA dependencies: `gate`
    # (the reused scratch tile) reads the xt1 transfer (RAW), and a stub
    # write into each bulk slice's destination (WAW) forces every bulk DMA
    # behind `gate`.
    gate = scratch
    nc.gpsimd.tensor_scalar_add(out=gate, in0=xts0[2][:, 0:1], scalar1=0.0)
    for ws, we in ((2, 6), (6, 14), (14, 22), (22, 30), (30, K_SPLINE)):
        nc.gpsimd.tensor_scalar_add(out=w_tile[:, ws, 0:1], in0=gate,
                                    scalar1=0.0)
        nc.gpsimd.dma_start(out=w_tile[:, ws:we, :], in_=w_sp[:, ws:we, :])

    psums = psums0
    for ch in range(n_chunks):
        emit_main_sweep(ch, psums, *prod)
        cur_prod, cur_psums = prod, psums
        if ch + 1 < n_chunks:
            prod = emit_producers(ch + 1)
            psums = alloc_psums(ch + 1)
        emit_final_blocks(ch, cur_psums, *cur_prod)


def build_program(b_shard=B_SHARD, b_chunk=B_CHUNK):
    nc = bacc.Bacc("TRN2", target_bir_lowering=False, debug=False,
                   num_devices=N_CORES)
    x16t = nc.dram_tensor("x16t", [IN_DIM, b_shard], F16,
                          kind="ExternalInput").ap()
    w_sp = nc.dram_tensor("w_sp", [128, K_SPLINE, OUT_DIM], F16,
                          kind="ExternalInput").ap()
    w_b = nc.dram_tensor("w_base", [128, K_BASE, OUT_DIM], F16,
                         kind="ExternalInput").ap()
    n_par = N_DT + N_DT * G
    params = nc.dram_tensor("params", [128, n_par], F32,
                            kind="ExternalInput").ap()
    out_t = nc.dram_tensor("out_t", [OUT_DIM, b_shard], F16,
                           kind="ExternalOutput").ap()
    with tile.TileContext(nc) as tc:
        _body(tc, x16t, w_sp, w_b, params, out_t, b_shard, b_chunk)
    nc.compile()
    return nc


def make_in_maps(x, gamma, beta, moving_mean, moving_var, W_spline, b_spline,
                 W_base, b_base, n_cores=N_CORES):
    """Host-side preprocessing + per-core input shards."""
    x = np.asarray(x, dtype=np.float32)
    gamma = np.asarray(gamma, dtype=np.float32)
    beta = np.asarray(beta, dtype=np.float32)
    moving_mean = np.asarray(moving_mean, dtype=np.float32)
    moving_var = np.asarray(moving_var, dtype=np.float32)
    W_spline = np.asarray(W_spline, dtype=np.float32)
    W_base = np.asarray(W_base, dtype=np.float32)

    scale = gamma / np.sqrt(moving_var + np.float32(BN_EPS))
    shift = beta - moving_mean * scale
    uscale = (scale / np.float32(DENOM)).astype(np.float32)
    ushift = (shift / np.float32(DENOM)).astype(np.float32)

    x16t = np.ascontiguousarray(x.T.astype(np.float16))  # [D, B]
    # K-order on chip is (dt, g, d_in): kc = dt*8+g covers d in
    # [dt*128, (dt+1)*128) at grid g.  W_spline rows are (d, g)-ordered.
    w_r = (W_spline.reshape(N_DT, 128, G, OUT_DIM)
           .transpose(0, 2, 1, 3)            # (dt, g, d_in, o)
           .reshape(K_SPLINE, 128, OUT_DIM)
           .transpose(1, 0, 2))              # (d_in, kc, o)
    w_sp = np.ascontiguousarray(w_r * np.float32(np.sqrt(np.pi) / 2.0)
                               ).astype(np.float16)
    w_b = np.ascontiguousarray(
        W_base.reshape(K_BASE, 128, OUT_DIM).transpose(1, 0, 2)
    ).astype(np.float16)
    c = _grid_consts()
    params = np.empty((128, N_DT + N_DT * G), np.float32)
    params[:, 0:N_DT] = uscale.reshape(N_DT, 128).T
    # ACT bias for (dt, g): ushift_d - c_g
    ub = ushift.reshape(N_DT, 128)            # [dt, 128]
    for dt in range(N_DT):
        for g in range(G):
            params[:, N_DT + dt * G + g] = ub[dt] - c[g]

    b_shard = x.shape[0] // n_cores
    return [
        {
            "x16t": np.ascontiguousarray(
                x16t[:, ci * b_shard:(ci + 1) * b_shard]),
            "w_sp": w_sp,
            "w_base": w_b,
            "params": params,
        }
        for ci in range(n_cores)
    ]


def gather_out(results, x, b_spline, b_base, n_cores=N_CORES):
    """Host-side unshard: transpose back, add residual + biases, fp32."""
    x = np.asarray(x, dtype=np.float32)
    bias = (np.asarray(b_spline, dtype=np.float32)
            + np.asarray(b_base, dtype=np.float32))
    out = np.concatenate(
        [results[ci]["out_t"].T.astype(np.float32)
         for ci in range(n_cores)], axis=0)
    out += x
    out += bias[None, :]
    return out


_PROGRAM = None


def kernel(x, gamma, beta, moving_mean, moving_var, W_spline, b_spline,
           W_base, b_base):
    global _PROGRAM
    if _PROGRAM is None:
        _PROGRAM = build_program()
    in_maps = make_in_maps(x, gamma, beta, moving_mean, moving_var,
                           W_spline, b_spline, W_base, b_base)
    res = run_bass_kernel_spmd(_PROGRAM, in_maps, core_ids=list(range(N_CORES)))
    return gather_out(res.results, x, b_spline, b_base)


# revision 36
# speedup vs baseline: 1.0063x; 1.0063x over previous
"""Trainium2 Bass kernel for EnhancedFastKANLayer.

Reference computation (B=16384, D=O=512, G=8 grids):
    x_norm = (x - mean) * rsqrt(var + eps) * gamma + beta          # BN inference
    basis[b,d,g] = exp(-((x_norm[b,d] - grid[g]) / denom)^2)       # RBF expansion
    out = basis.reshape(B, D*G) @ W_spline + b_spline
        + relu(x) @ W_base + b_base + x

Strategy (v3):
  - Data parallel: batch 16384 sharded 8 ways (2048 rows/core); weights
    replicated. No collectives.
  - All on-chip compute in transposed layout [feature, batch]; output is
    produced as out_T [O, B_shard] fp16 and combined on the host: the host
    adds the +x residual and the (b_spline+b_base) bias during the gather
    pass, so the chip runs only the 36 real K-chunks (4 base + 32 spline).
  - RBF via ScalarE Derivative_Erf: basis_g = sqrt(pi)/2 *
    Derivative_Erf(uscale_d*x + (ushift_d - c_g)): ONE ACT op per
    (d-tile, grid) directly from the fp16 x tile (BN affine folded into
    ACT's per-partition scale/bias; sqrt(pi)/2 folded into W_spline).
  - x pre-cast to fp16 and pre-transposed to [D, B_shard] on the host.
  - Matmul: kc-OUTER sweep; the 4 BASE K-chunks go FIRST -- relu(x) needs
    only the x tile (no params, no ACT), so the real matmul stream starts
    ~1.5us after the first x DMA lands, and the spline/weight DMA deadlines
    all relax by the ~7us the base chunks take.  Each basis tile is dead
    right after its 8 matmuls (small pool).  The last 6 spline K-chunks run
    per-bank, fused with the PSUM->SBUF fp16 copy + out DMA so bank
    epilogues pipeline with remaining matmuls.
  - Startup: critical-first DMA order (sync: x d-tile 0 in halves and
    W_base d-slice 0; gpsimd: ACT params then the bulk W_spline stream);
    N=512 warmup matmuls on a zeroed tile bridge the HAM clock-throttle
    window until the first base matmul.
"""

import numpy as np
from contextlib import ExitStack

import concourse.bass as bass
import concourse.tile as tile
from concourse import bacc, mybir
from concourse._compat import with_exitstack
from concourse.bass_utils import run_bass_kernel_spmd

N_CORES = 8
BATCH, IN_DIM, OUT_DIM, G = 16384, 512, 512, 8
B_SHARD = BATCH // N_CORES          # 2048
B_CHUNK = 1024                      # batch columns processed per chunk
GRID_MIN, GRID_MAX, BN_EPS = -2.0, 2.0, 1e-3
DENOM = (GRID_MAX - GRID_MIN) / G   # 0.5
N_DT = IN_DIM // 128                # 4 d-tiles
K_SPLINE = N_DT * G                 # 32 spline K-chunks
K_BASE = N_DT                       # 4 base K-chunks
N_OSUB = OUT_DIM // 128             # 4 output partition tiles

F32 = mybir.dt.float32
F16 = mybir.dt.float16


def _grid_consts():
    grid = np.linspace(GRID_MIN, GRID_MAX, G, dtype=np.float32)
    c = (grid / np.float32(DENOM)).astype(np.float32)        # grid in u-units
    return c


@with_exitstack
def _body(ctx, tc, x16t, w_sp, w_b, params, out_t, b_shard, b_chunk):
    nc = tc.nc
    n_chunks = b_shard // b_chunk
    n_bh = b_chunk // 512            # 512-wide moving-operand slices
    # K-chunk schedule: 4 base chunks first, then 32 spline chunks; the
    # last N_FINAL spline chunks run per-bank fused with the epilogue.
    N_FINAL = 6
    sweep_kcs = list(range(K_SPLINE, K_SPLINE + K_BASE)) + \
        list(range(K_SPLINE - N_FINAL))
    final_kcs = list(range(K_SPLINE - N_FINAL, K_SPLINE))
    N_WARM = 6                       # N=512 junk matmuls to lift HAM throttle

    const_pool = ctx.enter_context(tc.tile_pool(name="const", bufs=1))
    w_pool = ctx.enter_context(tc.tile_pool(name="w", bufs=1))
    xt_pool = ctx.enter_context(tc.tile_pool(name="xt", bufs=6))
    basis_pool = ctx.enter_context(tc.tile_pool(name="basis", bufs=8))
    relu_pool = ctx.enter_context(tc.tile_pool(name="relu", bufs=4))
    psum_pool = ctx.enter_context(
        tc.tile_pool(name="psum", bufs=8, space="PSUM"))
    out_pool = ctx.enter_context(tc.tile_pool(name="outs", bufs=3))

    # ---- ACT table preload: walrus inserts the ACT_TABLE_LOAD before this
    # first ACTIVATE so it overlaps the input DMAs ----
    # zeros tile: feeds the PE warm-up matmuls, and its first column doubles
    # as the scratch input for the ACT-table preload; the scratch output
    # tile is later reused as the bulk-DMA gate (WAW-ordered).
    zeros = const_pool.tile([128, 512], F16)
    nc.vector.memset(zeros, 0.0)
    scratch = const_pool.tile([128, 1], F16)
    nc.scalar.activation(out=scratch, in_=zeros[:, 0:1],
                         func=mybir.ActivationFunctionType.Derivative_Erf)

    # ---- critical-first DMA order.
    # sync (HWDGE): x d-tile 0 in halves + W_base d-slice 0, then the rest
    # of W_base, the remaining x tiles and W_spline K-chunks 0-1.
    # gpsimd (SWDGE): ACT params, then the bulk W_spline stream (emitted
    # after chunk-0 producers so its issues don't crowd the criticals).
    params_sb = const_pool.tile([128, N_DT + N_DT * G], F32)
    nc.gpsimd.dma_start(out=params_sb, in_=params)
    uscale_sb = params_sb[:, 0:N_DT]                    # per-d ACT scale
    abias_sb = params_sb[:, N_DT:]                      # per-(d,g) ACT bias
    # single weight tile: spline K-chunks 0..31, base chunks at 32..35
    w_all = w_pool.tile([128, K_SPLINE + K_BASE, OUT_DIM], F16)
    w_tile = w_all[:, 0:K_SPLINE, :]
    wb_tile = w_all[:, K_SPLINE:, :]

    # sync/HWDGE queue, ordered by consumption deadline (SWDGE latency is
    # ~5us, so every startup-critical transfer must ride the sync FIFO)
    xt0 = xt_pool.tile([128, b_chunk], F16, tag="xt")
    nc.sync.dma_start(out=xt0[:, 0:512], in_=x16t[0:128, 0:512])
    nc.sync.dma_start(out=wb_tile[:, 0:1, :], in_=w_b[:, 0:1, :])
    nc.sync.dma_start(out=xt0[:, 512:1024], in_=x16t[0:128, 512:1024])
    xt1 = xt_pool.tile([128, b_chunk], F16, tag="xt")
    nc.sync.dma_start(out=xt1, in_=x16t[128:256, 0:b_chunk])
    nc.sync.dma_start(out=wb_tile[:, 1:2, :], in_=w_b[:, 1:2, :])
    xt2 = xt_pool.tile([128, b_chunk], F16, tag="xt")
    nc.sync.dma_start(out=xt2, in_=x16t[256:384, 0:b_chunk])
    nc.sync.dma_start(out=wb_tile[:, 2:K_BASE, :], in_=w_b[:, 2:K_BASE, :])
    xt3 = xt_pool.tile([128, b_chunk], F16, tag="xt")
    nc.sync.dma_start(out=xt3, in_=x16t[384:512, 0:b_chunk])
    nc.sync.dma_start(out=w_tile[:, 0:2, :], in_=w_sp[:, 0:2, :])
    xts0 = [xt0, xt1, xt2, xt3]

    def emit_producers(ch, xts=None):
        b0 = ch * b_chunk
        relus, basis = [], []
        for dt in range(N_DT):
            if xts is not None:
                xt = xts[dt]
            else:
                xt = xt_pool.tile([128, b_chunk], F16, tag="xt")
                nc.sync.dma_start(
                    out=xt,
                    in_=x16t[dt * 128:(dt + 1) * 128, b0:b0 + b_chunk],
                )
            rl = relu_pool.tile([128, b_chunk], F16, tag="relu")
            if ch == 0 and dt == 0:
                # halves: the first base matmuls depend only on half 0
                nc.vector.tensor_scalar_max(
                    out=rl[:, 0:512], in0=xt[:, 0:512], scalar1=0.0)
                nc.vector.tensor_scalar_max(
                    out=rl[:, 512:1024], in0=xt[:, 512:1024], scalar1=0.0)
            else:
                nc.vector.tensor_scalar_max(out=rl, in0=xt, scalar1=0.0)
            relus.append(rl)
            for g in range(G):
                bt = basis_pool.tile([128, b_chunk], F16, tag="basis")
                # basis_g = sqrt(pi)/2 * d/dx erf(uscale*x + ushift - c_g);
                # constant folded into W_spline host-side.
                nc.scalar.activation(
                    out=bt, in_=xt,
                    func=mybir.ActivationFunctionType.Derivative_Erf,
                    bias=abias_sb[:, dt * G + g:dt * G + g + 1],
                    scale=uscale_sb[:, dt:dt + 1],
                )
                basis.append(bt)
        return relus, basis

    def operands(kc, osub, relus, basis):
        if kc < K_SPLINE:
            return w_tile[:, kc, osub * 128:(osub + 1) * 128], basis[kc]
        dt = kc - K_SPLINE
        return wb_tile[:, dt, osub * 128:(osub + 1) * 128], relus[dt]

    def emit_main_sweep(ch, psums, relus, basis):
        for idx, kc in enumerate(sweep_kcs):
            # bh-outer on the very first chunk's first K-chunk so the first
            # 4 matmuls depend only on the first relu half-tile.
            order = ([(osub, bh) for bh in range(n_bh)
                      for osub in range(N_OSUB)]
                     if (ch == 0 and idx == 0) else
                     [(osub, bh) for osub in range(N_OSUB)
                      for bh in range(n_bh)])
            for osub, bh in order:
                lhsT, rhs = operands(kc, osub, relus, basis)
                nc.tensor.matmul(
                    psums[osub * n_bh + bh], lhsT=lhsT,
                    rhs=rhs[:, bh * 512:(bh + 1) * 512],
                    start=(idx == 0), stop=False)

    def emit_final_blocks(ch, psums, relus, basis):
        b0 = ch * b_chunk
        for osub in range(N_OSUB):
            for bh in range(n_bh):
                ps = psums[osub * n_bh + bh]
                for j, kc in enumerate(final_kcs):
                    lhsT, rhs = operands(kc, osub, relus, basis)
                    nc.tensor.matmul(
                        ps, lhsT=lhsT, rhs=rhs[:, bh * 512:(bh + 1) * 512],
                        start=False, stop=(j == N_FINAL - 1))
                ot = out_pool.tile([128, 512], F16, tag="ot")
                nc.vector.tensor_scalar_add(out=ot, in0=ps, scalar1=0.0)
                nc.sync.dma_start(
                    out=out_t[osub * 128:(osub + 1) * 128,
                              b0 + bh * 512:b0 + (bh + 1) * 512],
                    in_=ot)

    def alloc_psums(ch):
        return [psum_pool.tile([128, 512], F32, tag="ps", name=f"ps{ch}_{i}")
                for i in range(N_OSUB * n_bh)]

    psums0 = alloc_psums(0)
    # PE warm-up: dependency-free matmuls release the HAM clock throttle
    # (~3.4us of sustained PE activity) while the first x tile lands; they
    # target the LAST-consumed bank, and the real first matmul there has
    # start=True which resets it, so junk never reaches the output.
    for j in range(N_WARM):
        nc.tensor.matmul(psums0[N_OSUB * n_bh - 1], lhsT=zeros[:, 0:128],
                         rhs=zeros, start=(j == 0), stop=(j == N_WARM - 1))

    prod = emit_producers(0, xts0)
    # bulk W_spline stream (gpsimd), gated behind the last critical sync
    # transfer (W_spline K-chunks 0-1) so the DMA rings stay clear for the
    # latency-critical startup transfers; the bulk has ~10us of slack
    # before K-chunk 2 is consumed.  The Tile scheduler reorders engine
    # streams by readiness, so the gating must be DATA dependencies: `gate`
    # (the reused scratch tile) reads the xt1 transfer (RAW), and a stub
    # write into each bulk slice's destination (WAW) forces every bulk DMA
    # behind `gate`.
    gate = scratch
    nc.gpsimd.tensor_scalar_add(out=gate, in0=xts0[2][:, 0:1], scalar1=0.0)
    for ws, we in ((2, 6), (6, 14), (14, 22), (22, 30), (30, K_SPLINE)):
        nc.gpsimd.tensor_scalar_add(out=w_tile[:, ws, 0:1], in0=gate,
                                    scalar1=0.0)
        nc.gpsimd.dma_start(out=w_tile[:, ws:we, :], in_=w_sp[:, ws:we, :])

    psums = psums0
    for ch in range(n_chunks):
        emit_main_sweep(ch, psums, *prod)
        cur_prod, cur_psums = prod, psums
        if ch + 1 < n_chunks:
            prod = emit_producers(ch + 1)
            psums = alloc_psums(ch + 1)
        emit_final_blocks(ch, cur_psums, *cur_prod)


def build_program(b_shard=B_SHARD, b_chunk=B_CHUNK):
    nc = bacc.Bacc("TRN2", target_bir_lowering=False, debug=False,
                   num_devices=N_CORES)
    x16t = nc.dram_tensor("x16t", [IN_DIM, b_shard], F16,
                          kind="ExternalInput").ap()
    w_sp = nc.dram_tensor("w_sp", [128, K_SPLINE, OUT_DIM], F16,
                          kind="ExternalInput").ap()
    w_b = nc.dram_tensor("w_base", [128, K_BASE, OUT_DIM], F16,
                         kind="ExternalInput").ap()
    n_par = N_DT + N_DT * G
    params = nc.dram_tensor("params", [128, n_par], F32,
                            kind="ExternalInput").ap()
    out_t = nc.dram_tensor("out_t", [OUT_DIM, b_shard], F16,
                           kind="ExternalOutput").ap()
    with tile.TileContext(nc) as tc:
        _body(tc, x16t, w_sp, w_b, params, out_t, b_shard, b_chunk)
    nc.compile()
    return nc


def make_in_maps(x, gamma, beta, moving_mean, moving_var, W_spline, b_spline,
                 W_base, b_base, n_cores=N_CORES):
    """Host-side preprocessing + per-core input shards."""
    x = np.asarray(x, dtype=np.float32)
    gamma = np.asarray(gamma, dtype=np.float32)
    beta = np.asarray(beta, dtype=np.float32)
    moving_mean = np.asarray(moving_mean, dtype=np.float32)
    moving_var = np.asarray(moving_var, dtype=np.float32)
    W_spline = np.asarray(W_spline, dtype=np.float32)
    W_base = np.asarray(W_base, dtype=np.float32)

    scale = gamma / np.sqrt(moving_var + np.float32(BN_EPS))
    shift = beta - moving_mean * scale
    uscale = (scale / np.float32(DENOM)).astype(np.float32)
    ushift = (shift / np.float32(DENOM)).astype(np.float32)

    x16t = np.ascontiguousarray(x.T.astype(np.float16))  # [D, B]
    # K-order on chip is (dt, g, d_in): kc = dt*8+g covers d in
    # [dt*128, (dt+1)*128) at grid g.  W_spline rows are (d, g)-ordered.
    w_r = (W_spline.reshape(N_DT, 128, G, OUT_DIM)
           .transpose(0, 2, 1, 3)            # (dt, g, d_in, o)
           .reshape(K_SPLINE, 128, OUT_DIM)
           .transpose(1, 0, 2))              # (d_in, kc, o)
    w_sp = np.ascontiguousarray(w_r * np.float32(np.sqrt(np.pi) / 2.0)
                               ).astype(np.float16)
    w_b = np.ascontiguousarray(
        W_base.reshape(K_BASE, 128, OUT_DIM).transpose(1, 0, 2)
    ).astype(np.float16)
    c = _grid_consts()
    params = np.empty((128, N_DT + N_DT * G), np.float32)
    params[:, 0:N_DT] = uscale.reshape(N_DT, 128).T
    # ACT bias for (dt, g): ushift_d - c_g
    ub = ushift.reshape(N_DT, 128)            # [dt, 128]
    for dt in range(N_DT):
        for g in range(G):
            params[:, N_DT + dt * G + g] = ub[dt] - c[g]

    b_shard = x.shape[0] // n_cores
    return [
        {
            "x16t": np.ascontiguousarray(
                x16t[:, ci * b_shard:(ci + 1) * b_shard]),
            "w_sp": w_sp,
            "w_base": w_b,
            "params": params,
        }
        for ci in range(n_cores)
    ]


def gather_out(results, x, b_spline, b_base, n_cores=N_CORES):
    """Host-side unshard: transpose back, add residual + biases, fp32."""
    x = np.asarray(x, dtype=np.float32)
    bias = (np.asarray(b_spline, dtype=np.float32)
            + np.asarray(b_base, dtype=np.float32))
    out = np.concatenate(
        [results[ci]["out_t"].T.astype(np.float32)
         for ci in range(n_cores)], axis=0)
    out += x
    out += bias[None, :]
    return out


_PROGRAM = None


def kernel(x, gamma, beta, moving_mean, moving_var, W_spline, b_spline,
           W_base, b_base):
    global _PROGRAM
    if _PROGRAM is None:
        _PROGRAM = build_program()
    in_maps = make_in_maps(x, gamma, beta, moving_mean, moving_var,
                           W_spline, b_spline, W_base, b_base)
    res = run_bass_kernel_spmd(_PROGRAM, in_maps, core_ids=list(range(N_CORES)))
    return gather_out(res.results, x, b_spline, b_base)
